# revision 1
# baseline (speedup 1.0000x reference)
"""Trainium2 kernel for nn_Model_25056839205077 (sparse_attention).

Mathematical collapse: the reference applies ``masked_fill(mask, -max)``
where ``mask`` is True at IN-BOUNDS positions (faithful port of a sign bug
in the source model).  Consequently:

- interior windows (all 16 halo pixels in-bounds): every sim entry is
  ``-float32.max`` -> softmax is uniform 1/16 -> the attention output is the
  mean of v over the 4x4 halo window.  Final output per 2x2 query block is
  ``Wo @ Wv @ mean_{4x4}(x) + bo`` (identical for all 4 pixels).
- boundary windows (any out-of-bounds halo pixel): softmax concentrates on
  the OOB positions where v is exactly 0 -> output is exactly ``bo``.

So the whole module reduces to a 4x4/stride-2 box filter followed by one
512x512 matvec per window plus bias, with the boundary ring forced to bo.
Verified against the full reference to ~1e-6 relative error.

Sharding: data-parallel over (batch, image half) -> 8 shards.  Bottom-half
shards are vertically flipped on the host so a single SPMD program (which
masks local window-row 0) serves all cores; the box filter is symmetric so
flipping commutes with the compute.
"""

import numpy as np

_PROGRAMS = {}

B, C, H, W = 4, 512, 64, 64
GROUPS = 4   # 512 channels = 4 groups of 128 partitions
WHS = 16     # window rows per shard (half image)
WWS = 32     # window cols

ALL_STAGES = ("dma", "filt", "mm", "act", "out")


def _emit_body(nc, tc, pool, psum_pool, warm_pool, xs, mt, bo, out,
               stages=ALL_STAGES, mm_dtype="f32", warm=True):
    from concourse import mybir

    f32 = mybir.dt.float32

    # Uneven rounds: round 0 (whose outputs can't start before the input
    # phase drains anyway) takes more window rows; the last round (which
    # sets the tail and accumulates PE/DVE lag) takes fewer.
    CHUNKS = [(0, 5), (5, 4), (9, 4), (13, 3)]

    xt = pool.tile([128, GROUPS * 32 * W], f32)   # (p, g, lr32, c64)
    vt = pool.tile([128, GROUPS * 16 * W], f32)   # (p, g, u16, c64)
    qt = pool.tile([128, GROUPS * 16 * 31], f32)  # (p, g, u16, j31)
    st = pool.tile([128, GROUPS * 15 * 31], f32)  # (p, g, wh-1, j31)
    mts = pool.tile([128, GROUPS * 512], f32)     # (p, k, co512)
    bos = pool.tile([128, GROUPS], f32)
    # Per-round tiles so round r+1's writes never alias round r's readers
    # (a shared tile serializes the dup stage behind the previous round's
    # output DMA).
    xms = [
        pool.tile([128, GROUPS * nw * WWS], f32, name=f"xm{c}", tag=f"xm{c}")
        for c, (w0, nw) in enumerate(CHUNKS)
    ]
    yds = [
        pool.tile([128, GROUPS * nw * 2 * W], f32, name=f"yd{c}", tag=f"yd{c}")
        for c, (w0, nw) in enumerate(CHUNKS)
    ]

    xtv = xt[:].rearrange("p (g r c) -> p g r c", g=GROUPS, r=32)
    vtv = vt[:].rearrange("p (g u c) -> p g u c", g=GROUPS, u=16)
    qtv = qt[:].rearrange("p (g u j) -> p g u j", g=GROUPS, u=16)
    stv = st[:].rearrange("p (g w j) -> p g w j", g=GROUPS, w=15)
    mtv = mts[:].rearrange("p (k co) -> p k co", k=GROUPS)
    xmvs = [
        t[:].rearrange("p (g w c) -> p g w c", g=GROUPS, w=CHUNKS[i][1])
        for i, t in enumerate(xms)
    ]
    ydvs = [
        t[:].rearrange("p (co w dr ww dc) -> p co w dr ww dc",
                       co=GROUPS, w=CHUNKS[i][1], dr=2, ww=WWS)
        for i, t in enumerate(yds)
    ]

    xsv = xs.ap().rearrange("(g p) r c -> p g r c", p=128)
    mtdv = mt.ap().rearrange("(k p) co -> p k co", p=128)
    outv = out.ap().rearrange("(g p) r c -> p g r c", p=128)

    scratch = pool.tile([128, 512], f32)
    nc.gpsimd.memset(scratch[:, :], 0.0)

    # Trigger the one-time ACT Identity-table load (~1.3us) during the DMA
    # head instead of in front of the first real bias-add.
    nc.scalar.add(scratch[:, 0:1], scratch[:, 1:2], 0.0)

    # Boundary windows must produce exactly bo: zero their xmean so the
    # matmul contributes nothing there (window row 0 and columns 0/31).
    nc.gpsimd.memset(xmvs[0][:, :, 0:1, :], 0.0)
    for c in range(len(CHUNKS)):
        nc.gpsimd.memset(xmvs[c][:, :, :, 0:1], 0.0)
        nc.gpsimd.memset(xmvs[c][:, :, :, WWS - 1 : WWS], 0.0)

    # All input traffic: x chunks + weights on the sync HWDGE ring in
    # priority order (the first x chunk unblocks the filter pipeline, the
    # weights are needed by the first matmul round).  bo goes via SWDGE so
    # it doesn't occupy a sync-ring issue slot ahead of x0.  Input x rows
    # 1..32 are stored at tile rows 0..31.
    nc.gpsimd.dma_start(out=bos[:, :], in_=bo.ap())
    if "dma" in stages:
        e0 = 2 * (CHUNKS[0][0] + CHUNKS[0][1])
        nc.sync.dma_start(out=xtv[:, :, 0:e0, :], in_=xsv[:, :, 0:e0, :])
    # The weights come in two halves with x1 in between: round 0 only needs
    # k-tiles 0-1 while its first half runs, and the split pulls x1 (and so
    # filter 1 -> round 1 -> its output DMA) ~1.5us earlier.
    nc.sync.dma_start(out=mtv[:, 0:2, :], in_=mtdv[:, 0:2, :])
    if "dma" in stages:
        w0, nw = CHUNKS[1]
        nc.sync.dma_start(out=xtv[:, :, 2 * w0 : 2 * (w0 + nw), :],
                          in_=xsv[:, :, 2 * w0 : 2 * (w0 + nw), :])
    nc.sync.dma_start(out=mtv[:, 2:4, :], in_=mtdv[:, 2:4, :])
    if "dma" in stages:
        for (w0, nw) in CHUNKS[2:]:
            nc.sync.dma_start(
                out=xtv[:, :, 2 * w0 : 2 * (w0 + nw), :],
                in_=xsv[:, :, 2 * w0 : 2 * (w0 + nw), :],
            )

    # PE warm-up + inter-round fillers: bf16 matmuls on a cast of the first
    # x chunk.  The data dependency on the x0 DMA makes the warm-up start
    # right before the first real round; fillers between rounds keep the PE
    # busy across the short waits for the next filter chunk.  Both the HAM
    # clock-gate and the cost model throttle the PE clock after idle gaps,
    # which would otherwise double every matmul's cost.
    wsrc = None
    if warm and "mm" in stages and "dma" in stages:
        wsrc = pool.tile([128, 512], mybir.dt.bfloat16)
        nc.scalar.copy(wsrc[:, :], xtv[:, 0, 0:8, :].rearrange("p a b -> p (a b)"))
        wps = warm_pool.tile([128, 512], f32)
        for _ in range(4):
            nc.tensor.matmul(wps[:, :], wsrc[:, 0:128], wsrc[:, :],
                             start=True, stop=True)

    # Separable 4x4/stride-2 box filter via pairwise sums:
    #   V[u]     = x[2u+1] + x[2u+2]          (vertical pairs)
    #   Q[u, j]  = V[u, 2j+1] + V[u, 2j+2]     (horizontal pairs)
    #   S[wh, j] = Q[wh-1, j] + Q[wh, j]       (vertical final)
    #   xm[wh,ww]= S[wh, ww-1] + S[wh, ww]     (horizontal final)
    # The 1/16 is folded into mt on the host.
    def emit_filter(c):
        u0, u1 = CHUNKS[c][0], CHUNKS[c][0] + CHUNKS[c][1]
        wlo = max(u0, 1)
        nc.vector.tensor_add(
            vtv[:, :, u0:u1, :],
            xtv[:, :, 2 * u0 : 2 * u1 : 2, :],
            xtv[:, :, 2 * u0 + 1 : 2 * u1 : 2, :],
        )
        # Q on GpSimd: takes one add off the DVE, which is otherwise the
        # busiest engine in steady state.
        nc.gpsimd.tensor_add(
            qtv[:, :, u0:u1, :],
            vtv[:, :, u0:u1, 1:62:2],
            vtv[:, :, u0:u1, 2:63:2],
        )
        nc.vector.tensor_add(
            stv[:, :, wlo - 1 : u1 - 1, :],
            qtv[:, :, wlo - 1 : u1 - 1, :],
            qtv[:, :, wlo:u1, :],
        )
        nc.vector.tensor_add(
            xmvs[c][:, :, wlo - u0 : u1 - u0, 1:31],
            stv[:, :, wlo - 1 : u1 - 1, 0:30],
            stv[:, :, wlo - 1 : u1 - 1, 1:31],
        )

    def emit_round(c):
        nw = CHUNKS[c][1]
        for co in range(GROUPS):
            ps = psum_pool.tile([128, nw * WWS], f32, name=f"ps{c}_{co}", tag="ps")
            for k in range(GROUPS):
                nc.tensor.matmul(
                    ps[:, :],
                    mtv[:, k, 128 * co : 128 * co + 128],
                    xmvs[c][:, k, :, :],
                    start=(k == 0),
                    stop=(k == GROUPS - 1),
                )
            # Bias-add + duplicate each window value into its 2x2 output
            # block.  ISA activation patterns cap at 3 free dims, so one op
            # per output row (dr).  co 0-2 on ACT, co 3 on DVE — legal here
            # because the pipelined emission order puts the next filter
            # chunk's DVE ops ahead of this round's DVE dup, so nothing
            # ready is stuck behind the PE wait.
            psb = (
                ps[:]
                .rearrange("p (w ww) -> p w ww", w=nw)
                .unsqueeze(3)
                .broadcast_to((128, nw, WWS, 2))
            )
            if "act" in stages:
                for dr in range(2):
                    dst = ydvs[c][:, co, :, dr, :, :]
                    if co < 3:
                        nc.scalar.add(dst, psb, bos[:, co : co + 1])
                    else:
                        nc.vector.tensor_scalar_add(dst, psb, bos[:, co : co + 1])
            # Half-round output DMAs (after co1 and co3): transfers start as
            # soon as half the dups are done, packing the DMA engines
            # earlier.  Issued on the scalar HWDGE ring, off the input FIFO.
            if "out" in stages:
                w0 = CHUNKS[c][0]
                ydr = yds[c][:].rearrange("p (co f) -> p co f", co=GROUPS)
                nc.sync.dma_start(
                    out=outv[:, co : co + 1, 2 * w0 : 2 * (w0 + nw), :],
                    in_=ydr[:, co : co + 1, :],
                )
        # No inter-round filler matmuls: the HW HAM clock-gate tolerates the
        # sub-1.5us waits between rounds (its idle window is ~3.4us), so
        # fillers only lengthen the PE queue.  (The cost model disagrees —
        # it re-throttles on any gap — but HW is the ground truth here.)

    # Software-pipelined emission: the filter for chunk c+1 is emitted ahead
    # of round c so per-engine instruction streams stay dependency-monotone
    # (each engine's next instruction is never waiting on a producer that is
    # further downstream than what follows it).
    if "filt" in stages:
        emit_filter(0)
    for c in range(len(CHUNKS)):
        if "filt" in stages and c < len(CHUNKS) - 1:
            emit_filter(c + 1)
        if "mm" in stages:
            emit_round(c)


def _build_program(iters=1, stages=ALL_STAGES, mm_dtype="f32", warm=True):
    import concourse.tile as tile
    from concourse import bacc, mybir

    f32 = mybir.dt.float32
    nc = bacc.Bacc("TRN2", target_bir_lowering=False, debug=False)

    xs = nc.dram_tensor("xs", (C, 32, W), f32, kind="ExternalInput")
    mt = nc.dram_tensor("mt", (C, C), f32, kind="ExternalInput")
    bo = nc.dram_tensor("bo_t", (128, GROUPS), f32, kind="ExternalInput")
    out = nc.dram_tensor("out", (C, 32, W), f32, kind="ExternalOutput")

    with tile.TileContext(nc) as tc:
        with (
            tc.tile_pool(name="main", bufs=1) as pool,
            tc.tile_pool(name="psum", bufs=7, space="PSUM") as psum_pool,
            tc.tile_pool(name="warmps", bufs=1, space="PSUM") as warm_pool,
        ):
            for _ in range(iters):
                _emit_body(nc, tc, pool, psum_pool, warm_pool, xs, mt, bo, out,
                           stages, mm_dtype, warm)

    nc.compile()
    return nc


def _get_program(iters=1, stages=ALL_STAGES, mm_dtype="f32", warm=True):
    key = (iters, tuple(stages), mm_dtype, warm)
    if key not in _PROGRAMS:
        _PROGRAMS[key] = _build_program(iters, stages, mm_dtype, warm)
    return _PROGRAMS[key]


def _host_prep(x, Wkv, Wo, bo):
    x = np.asarray(x, dtype=np.float32)
    Wkv = np.asarray(Wkv, dtype=np.float32)
    Wo = np.asarray(Wo, dtype=np.float32)
    bo = np.asarray(bo, dtype=np.float32)
    M = (Wo @ Wkv[C:]).astype(np.float32)
    mt = np.ascontiguousarray(M.T * np.float32(1.0 / 16.0))
    bo_t = np.ascontiguousarray(bo.reshape(GROUPS, 128).T)
    shards = []
    for core in range(8):
        b, half = core // 2, core % 2
        if half == 0:
            xsh = x[b, :, 1:33, :]
        else:
            xsh = x[b, :, 62:30:-1, :]
        shards.append(np.ascontiguousarray(xsh))
    return shards, mt, bo_t


def _gather(results):
    out = np.empty((B, C, H, W), dtype=np.float32)
    for core in range(8):
        r = np.asarray(results[core]["out"])
        b, half = core // 2, core % 2
        if half == 0:
            out[b, :, 0:32, :] = r
        else:
            out[b, :, 32:64, :] = r[:, ::-1, :]
    return out


def kernel(x, Wq, Wkv, Wo, bo, _trace=False, _iters=1, _mm_dtype="f32"):
    from concourse.bass_utils import run_bass_kernel_spmd

    shards, mt, bo_t = _host_prep(x, Wkv, Wo, bo)
    nc = _get_program(_iters, mm_dtype=_mm_dtype)
    in_maps = [{"xs": s, "mt": mt, "bo_t": bo_t} for s in shards]
    res = run_bass_kernel_spmd(nc, in_maps, list(range(8)), trace=_trace)
    out = _gather(res.results)
    if _trace:
        return out, res
    return out



# revision 12
# speedup vs baseline: 3.1295x; 3.1295x over previous
"""Trainium2 kernel for nn_Model_25056839205077 (sparse_attention).

Mathematical collapse: the reference applies ``masked_fill(mask, -max)``
where ``mask`` is True at IN-BOUNDS positions (faithful port of a sign bug
in the source model).  Consequently:

- interior windows (all 16 halo pixels in-bounds): every sim entry is
  ``-float32.max`` -> softmax is uniform 1/16 -> the attention output is the
  mean of v over the 4x4 halo window.  Final output per 2x2 query block is
  ``Wo @ Wv @ mean_{4x4}(x) + bo`` (identical for all 4 pixels).
- boundary windows (any out-of-bounds halo pixel): softmax concentrates on
  the OOB positions where v is exactly 0 -> output is exactly ``bo``.

So the whole module reduces to a 4x4/stride-2 box filter followed by one
512x512 matvec per interior window plus bias, with the boundary ring forced
to bo.

This version moves the bare minimum of bytes (the previous f32 full-output
kernel ran at the DMA fabric roofline, ~440 GB/s):

- everything on device is fp16 (~35x error margin vs the 2e-2 gate);
- the device computes ONLY the 15x30 interior windows of its half-image
  shard and writes ONE value per window; the host expands each value to
  its 2x2 output block and fills the boundary ring with bo during the
  gather (pure layout, no arithmetic);
- per-core traffic drops 9.4 MB -> ~3.0 MB (x 2.1 MB + folded weights
  0.5 MB + out 0.44 MB).

Sharding: data-parallel over (batch, image half) -> 8 shards.  Bottom-half
shards are vertically flipped on the host so a single SPMD program serves
all cores; the box filter is symmetric so flipping commutes.
"""

import numpy as np

_PROGRAMS = {}

B, C, H, W = 4, 512, 64, 64
GROUPS = 4   # 512 channels = 4 groups of 128 partitions
NWH = 15     # interior window rows per half-image shard
NWW = 30     # interior window cols

ALL_STAGES = ("dma", "filt", "mm", "act", "out")

# (w0, nw) window-row chunks (w in 1..15).  Chunk c computes NEW V/Q rows
# [u0, w0+nw) with u0 = 0 for c==0 else w0, i.e. x rows [2*u0, 2*(w0+nw)).
# Window w's S row needs Q[w-1] and Q[w]; Q[w0-1] comes from the previous
# chunk, so chunks are disjoint in V rows and cover all of x.
CHUNKS = [(1, 5), (6, 4), (10, 3), (13, 3)]


def _emit_body(nc, tc, pool, psum_pool, warm_pool, xs, mt, bo, out,
               stages=ALL_STAGES, warm=True):
    from concourse import mybir

    f16 = mybir.dt.float16
    f32 = mybir.dt.float32

    xt = pool.tile([128, GROUPS * 32 * W], f16)   # (p, g, xrow32, c64)
    vt = pool.tile([128, GROUPS * 16 * W], f16)   # (p, g, u16, c64)
    qt = pool.tile([128, GROUPS * 16 * 31], f16)  # (p, g, u16, j31)
    st = pool.tile([128, GROUPS * NWH * 31], f16) # (p, g, w15, j31)
    mts = pool.tile([128, GROUPS * 512], f16)     # (p, k, co512)
    bos = pool.tile([128, GROUPS], f32)
    # Per-chunk tiles so chunk c+1's writes never alias chunk c's readers.
    xms = [
        pool.tile([128, GROUPS * nw * NWW], f16, name=f"xm{c}", tag=f"xm{c}")
        for c, (w0, nw) in enumerate(CHUNKS)
    ]
    # (w, co, ww) free order so the chunk out-DMA is contiguous per
    # partition in the (p, w, g, ww) DRAM layout: runs of nw*240 B.
    ocs = [
        pool.tile([128, nw * GROUPS * NWW], f16, name=f"oc{c}", tag=f"oc{c}")
        for c, (w0, nw) in enumerate(CHUNKS)
    ]

    xtv = xt[:].rearrange("p (g r c) -> p g r c", g=GROUPS, r=32)
    vtv = vt[:].rearrange("p (g u c) -> p g u c", g=GROUPS, u=16)
    qtv = qt[:].rearrange("p (g u j) -> p g u j", g=GROUPS, u=16)
    stv = st[:].rearrange("p (g w j) -> p g w j", g=GROUPS, w=NWH)
    mtv = mts[:].rearrange("p (k co) -> p k co", k=GROUPS)
    xmvs = [
        t[:].rearrange("p (g w c) -> p g w c", g=GROUPS, w=CHUNKS[i][1])
        for i, t in enumerate(xms)
    ]
    ocvs = [
        t[:].rearrange("p (w co ww) -> p w co ww", w=CHUNKS[i][1], co=GROUPS)
        for i, t in enumerate(ocs)
    ]

    xsv = xs.ap().rearrange("(g p) r c -> p g r c", p=128)
    mtdv = mt.ap().rearrange("(k p) co -> p k co", p=128)
    outv = out.ap()  # (p, w, g, ww) partition-major

    scratch = pool.tile([128, 512], f32)
    nc.gpsimd.memset(scratch[:, :], 0.0)

    # Trigger the one-time ACT Identity-table load (~1.3us) during the DMA
    # head instead of in front of the first real bias-add.
    nc.scalar.add(scratch[:, 0:1], scratch[:, 1:2], 0.0)

    # bo via SWDGE so it doesn't occupy a sync-ring issue slot ahead of x.
    nc.gpsimd.dma_start(out=bos[:, :], in_=bo.ap())

    # Input traffic on the sync HWDGE ring in priority order: a tiny x
    # prefix (rows 0-1) to unblock the PE warm-up, then x chunks
    # interleaved with the two weight halves (round 0 only needs k-tiles
    # 0-1).
    if "dma" in stages:
        nc.sync.dma_start(out=xtv[:, :, 0:2, :], in_=xsv[:, :, 0:2, :])
        e0 = 2 * (CHUNKS[0][0] + CHUNKS[0][1])
        nc.sync.dma_start(out=xtv[:, :, 2:e0, :], in_=xsv[:, :, 2:e0, :])
    nc.sync.dma_start(out=mtv[:, 0:2, :], in_=mtdv[:, 0:2, :])
    if "dma" in stages:
        w0, nw = CHUNKS[1]
        r0, r1 = 2 * w0, 2 * (w0 + nw)
        nc.sync.dma_start(out=xtv[:, :, r0:r1, :], in_=xsv[:, :, r0:r1, :])
    nc.sync.dma_start(out=mtv[:, 2:4, :], in_=mtdv[:, 2:4, :])
    if "dma" in stages:
        for (w0, nw) in CHUNKS[2:]:
            r0, r1 = 2 * w0, 2 * (w0 + nw)
            nc.sync.dma_start(out=xtv[:, :, r0:r1, :], in_=xsv[:, :, r0:r1, :])

    # PE warm-up: fp16 matmuls gated only on the tiny x prefix DMA, so the
    # HAM clock-gate sees activity through the DMA head and the real
    # matmuls run at 2.4 GHz.  (HW re-throttles only after ~3.4us idle, so
    # no inter-round fillers are needed.)
    if warm and "mm" in stages and "dma" in stages:
        wsrc = pool.tile([128, 512], f16)
        wsv = wsrc[:].rearrange("p (g r c) -> p g r c", g=GROUPS, r=2)
        nc.scalar.copy(wsv[:, :, :, :], xtv[:, :, 0:2, :])
        wps = warm_pool.tile([128, 512], f32)
        for _ in range(6):
            nc.tensor.matmul(wps[:, :], wsrc[:, 0:128], wsrc[:, :],
                             start=True, stop=True)

    # Separable 4x4/stride-2 box filter via pairwise sums (fp16 on DVE; the
    # stride-2 Q stage runs at 1x, the packed stages at 2x):
    #   V[u]     = x[2u] + x[2u+1]            u in [w0-1, w0+nw-1]
    #   Q[u, j]  = V[u, 2j+1] + V[u, 2j+2]    j in 0..30
    #   S[w, j]  = Q[w-1, j] + Q[w, j]        w in w0..w0+nw-1 (stored at w-1)
    #   xm[w,ww] = S[w, ww-1] + S[w, ww]      ww in 1..30 (stored at ww-1)
    # The 1/16 is folded into mt on the host.
    def emit_filter(c):
        w0, nw = CHUNKS[c]
        u0 = 0 if c == 0 else w0       # new V rows for this chunk
        u1 = w0 + nw
        nc.vector.tensor_add(
            vtv[:, :, u0:u1, :],
            xtv[:, :, 2 * u0 : 2 * u1 : 2, :],
            xtv[:, :, 2 * u0 + 1 : 2 * u1 : 2, :],
        )
        nc.vector.tensor_add(
            qtv[:, :, u0:u1, :],
            vtv[:, :, u0:u1, 1:62:2],
            vtv[:, :, u0:u1, 2:63:2],
        )
        nc.vector.tensor_add(
            stv[:, :, w0 - 1 : w0 + nw - 1, :],
            qtv[:, :, w0 - 1 : w0 + nw - 1, :],
            qtv[:, :, w0 : w0 + nw, :],
        )
        nc.vector.tensor_add(
            xmvs[c][:, :, :, :],
            stv[:, :, w0 - 1 : w0 + nw - 1, 0:30],
            stv[:, :, w0 - 1 : w0 + nw - 1, 1:31],
        )

    def emit_round(c):
        w0, nw = CHUNKS[c]
        for co in range(GROUPS):
            ps = psum_pool.tile([128, nw * NWW], f32, name=f"ps{c}_{co}", tag="ps")
            for k in range(GROUPS):
                nc.tensor.matmul(
                    ps[:, :],
                    mtv[:, k, 128 * co : 128 * co + 128],
                    xmvs[c][:, k, :, :].rearrange("p a b -> p (a b)"),
                    start=(k == 0),
                    stop=(k == GROUPS - 1),
                )
            # Bias add + f32->fp16 cast on ACT (otherwise idle).
            if "act" in stages:
                nc.scalar.add(
                    ocvs[c][:, :, co, :],
                    ps[:].rearrange("p (w ww) -> p w ww", w=nw),
                    bos[:, co : co + 1],
                )
        # Output DMA on the ACT HWDGE ring, right after its producer.
        if "out" in stages:
            nc.scalar.dma_start(
                out=outv[:, w0 - 1 : w0 + nw - 1, :, :],
                in_=ocvs[c][:, :, :, :],
            )

    # Software-pipelined emission: the filter for chunk c+1 is emitted ahead
    # of round c so per-engine instruction streams stay dependency-monotone.
    if "filt" in stages:
        emit_filter(0)
    for c in range(len(CHUNKS)):
        if "filt" in stages and c < len(CHUNKS) - 1:
            emit_filter(c + 1)
        if "mm" in stages:
            emit_round(c)


def _build_program(iters=1, stages=ALL_STAGES, warm=True):
    import concourse.tile as tile
    from concourse import bacc, mybir

    f16 = mybir.dt.float16
    f32 = mybir.dt.float32
    nc = bacc.Bacc("TRN2", target_bir_lowering=False, debug=False)

    xs = nc.dram_tensor("xs", (C, 32, W), f16, kind="ExternalInput")
    mt = nc.dram_tensor("mt", (C, C), f16, kind="ExternalInput")
    bo = nc.dram_tensor("bo_t", (128, GROUPS), f32, kind="ExternalInput")
    out = nc.dram_tensor("out", (128, NWH, GROUPS, NWW), f16,
                         kind="ExternalOutput")

    with tile.TileContext(nc) as tc:
        with (
            tc.tile_pool(name="main", bufs=1) as pool,
            tc.tile_pool(name="psum", bufs=7, space="PSUM") as psum_pool,
            tc.tile_pool(name="warmps", bufs=1, space="PSUM") as warm_pool,
        ):
            for _ in range(iters):
                _emit_body(nc, tc, pool, psum_pool, warm_pool, xs, mt, bo, out,
                           stages, warm)

    nc.compile()
    return nc


def _get_program(iters=1, stages=ALL_STAGES, warm=True):
    key = (iters, tuple(stages), warm)
    if key not in _PROGRAMS:
        _PROGRAMS[key] = _build_program(iters, stages, warm)
    return _PROGRAMS[key]


def _host_prep(x, Wkv, Wo, bo):
    x = np.asarray(x, dtype=np.float32)
    Wkv = np.asarray(Wkv, dtype=np.float32)
    Wo = np.asarray(Wo, dtype=np.float32)
    bo = np.asarray(bo, dtype=np.float32)
    M = (Wo @ Wkv[C:]).astype(np.float32)
    mt = np.ascontiguousarray((M.T * np.float32(1.0 / 16.0)).astype(np.float16))
    bo_t = np.ascontiguousarray(bo.reshape(GROUPS, 128).T)
    shards = []
    for core in range(8):
        b, half = core // 2, core % 2
        if half == 0:
            xsh = x[b, :, 1:33, :]
        else:
            xsh = x[b, :, 62:30:-1, :]
        shards.append(np.ascontiguousarray(xsh.astype(np.float16)))
    return shards, mt, bo_t


def _gather(results, bo):
    bo = np.asarray(bo, dtype=np.float32)
    out = np.empty((B, C, H, W), dtype=np.float32)
    # Boundary ring (windows touching the image border) is exactly bo.
    bcast = bo[None, :, None, None]
    out[:, :, 0:2, :] = bcast
    out[:, :, 62:64, :] = bcast
    out[:, :, 2:62, 0:2] = bcast
    out[:, :, 2:62, 62:64] = bcast
    for core in range(8):
        r = np.asarray(results[core]["out"])  # (128, 15, 4, 30) fp16
        # channel c = g*128 + p
        r = np.transpose(r, (2, 0, 1, 3)).reshape(C, NWH, NWW).astype(np.float32)
        b, half = core // 2, core % 2
        if half == 1:
            r = r[:, ::-1, :]  # local w 1..15 <-> global wh 31-w
        # expand each window value to its 2x2 output block
        e = np.repeat(np.repeat(r, 2, axis=1), 2, axis=2)  # (C, 30, 60)
        if half == 0:
            out[b, :, 2:32, 2:62] = e
        else:
            out[b, :, 32:62, 2:62] = e
    return out


def kernel(x, Wq, Wkv, Wo, bo, _trace=False, _iters=1):
    from concourse.bass_utils import run_bass_kernel_spmd

    shards, mt, bo_t = _host_prep(x, Wkv, Wo, bo)
    nc = _get_program(_iters)
    in_maps = [{"xs": s, "mt": mt, "bo_t": bo_t} for s in shards]
    res = run_bass_kernel_spmd(nc, in_maps, list(range(8)), trace=_trace)
    out = _gather(res.results, bo)
    if _trace:
        return out, res
    return out
